# revision 28
# baseline (speedup 1.0000x reference)
"""AttnGRU Trainium2 kernel: 8-way data-parallel, H-major dataflow.

v5 over the baseline (same host I/O contract as the baseline):
  - The 32-batch scan is split into two 16-batch half-chains (A/B) with
    fully separate tiles (own PSUM tile, own hbf/h tiles, own ring
    regions), so the tile dep-tracker sees no overlap and the two
    recurrence chains pipeline against each other: while A's
    sigmoid/tanh chain runs, the PE runs B's matmuls, and vice versa.
  - Ring slots are laid out per half: [xrA|ubA | xrB|ubB | xnA|xnB]
    (768 cols), so each half's PSUM init is ONE contiguous identity
    matmul and each chain op reads compact APs.
  - 2-block precompute ring; block i+1 is emitted 2 m-chunks per scan
    step of block i (no PE bursts, WAR pre-satisfied); psum->ring copies
    run on the ACT engine with the bias add fused.
  - Short critical chain per half: q = gt*h runs off-chain during the
    matmuls; chain is sigmoid -> n2 -> an(Pool) -> tanh -> p -> hbf; the
    f32 h update happens in the chain's shadow.

Math per core (B_loc=32, T=128, H=1024):
  xr = x @ Wr_w.T + (Wr_b + Ur_b)      (precomputed, blocked over time)
  xn = x @ W_w.T  + W_b
  per step: rt = sigmoid(xr_t + h @ Ur_w.T)
            nt = tanh(xn_t + rt * (h @ U_w.T + U_b))
            h  = (1-gt)*nt + gt*h

Tiles are H-major: [128 partitions = H-chunk, free = (chunk, batch)].
"""

import numpy as np
import ml_dtypes

import concourse.bass as bass
import concourse.bacc as bacc
import concourse.mybir as mybir
from concourse import tile
from concourse.bass_utils import run_bass_kernel_spmd

B, T, H = 256, 128, 1024
NCORES = 8
BL = B // NCORES          # 32 batch rows per core
BT = BL * T               # 4096 (time-major: col = t*32 + b)
KC = H // 128             # 8 contraction chunks
MC = 2048 // 128          # 16 output chunks ([r | n] concat)
BLK = 8                   # scan steps per precompute block
NBLK = T // BLK           # 16
NRING = 2                 # ring depth in blocks
RING = NRING * BLK        # 16 per-step slots
SLOT = 768                # per-slot cols: [xrA 128|ubA 128|xrB 128|ubB 128|xnA 128|xnB 128]

BF = mybir.dt.bfloat16
F32 = mybir.dt.float32
AF = mybir.ActivationFunctionType
OP = mybir.AluOpType

_CACHE = {}


def _build_bass():
    nc = bacc.Bacc()
    xT = nc.declare_dram_parameter("xT", [H, BT], BF, isOutput=False)
    wpreT = nc.declare_dram_parameter("wpreT", [H, 2048], BF, isOutput=False)
    uuT = nc.declare_dram_parameter("uuT", [H, 2048], BF, isOutput=False)
    biasp = nc.declare_dram_parameter("biasp", [128, MC], F32, isOutput=False)
    ubT = nc.declare_dram_parameter("ubT", [128, 256], BF, isOutput=False)
    gtT = nc.declare_dram_parameter("gtT", [128, 256], F32, isOutput=False)
    h0T = nc.declare_dram_parameter("h0T", [128, 256], F32, isOutput=False)
    out = nc.declare_dram_parameter("out", [128, 256], F32, isOutput=True)

    with tile.TileContext(nc) as tc:
        with (
            tc.tile_pool(name="w", bufs=1) as wp,
            tc.tile_pool(name="ew", bufs=3) as ew,
            tc.tile_pool(name="ps", bufs=1, space="PSUM") as psp,
            tc.tile_pool(name="pp", bufs=1, space="PSUM") as ppp,
        ):
            xT_sb = [wp.tile([128, BT], BF, tag=f"xT{k}", name=f"xT{k}") for k in range(KC)]
            uu_sb = [wp.tile([128, 2048], BF, tag=f"uu{k}", name=f"uu{k}") for k in range(KC)]
            wpre_sb = [wp.tile([128, 2048], BF, tag=f"wp{k}", name=f"wp{k}") for k in range(KC)]
            ring = wp.tile([128, RING * SLOT], BF, tag="ring")
            bias_sb = wp.tile([128, MC], F32, tag="bias")
            ub_sb = wp.tile([128, 256], BF, tag="ub")
            gt_sb = wp.tile([128, 256], F32, tag="gt")
            # per-half (16-batch) compact tiles for the two pipelined chains
            gth = [wp.tile([128, 128], F32, tag=f"gth{h}", name=f"gth{h}") for h in range(2)]
            gtch = [wp.tile([128, 128], F32, tag=f"gtch{h}", name=f"gtch{h}") for h in range(2)]
            hTh = [wp.tile([128, 128], F32, tag=f"hTh{h}", name=f"hTh{h}") for h in range(2)]
            hbfh = [wp.tile([128, 128], BF, tag=f"hbfh{h}", name=f"hbfh{h}") for h in range(2)]

            def half3(ap, h, b=32):
                # [128, (c b)] view, half h -> [128, c, 16] (strided 16-col slices)
                v = ap.rearrange("p (c b) -> p c b", b=b)
                return v[:, :, h * 16:(h + 1) * 16]

            def comp3(tile_):
                # compact [128, 128] tile as [128, c, 16]
                return tile_[:, :].rearrange("p (c b) -> p c b", b=16)

            nc.sync.dma_start(out=gt_sb[:, :], in_=gtT[:, :])
            nc.sync.dma_start(out=bias_sb[:, :], in_=biasp[:, :])
            nc.sync.dma_start(out=ub_sb[:, :], in_=ubT[:, :])
            for h in range(2):
                nc.sync.dma_start(out=comp3(hTh[h]), in_=half3(h0T[:, :], h))
            XH = 2 * BLK * 32        # 512 cols: blocks 0-1
            for k in range(KC):
                nc.scalar.dma_start(out=wpre_sb[k][:, :], in_=wpreT[k * 128:(k + 1) * 128, :])
                nc.sync.dma_start(out=xT_sb[k][:, 0:XH], in_=xT[k * 128:(k + 1) * 128, 0:XH])
            for k in range(KC):
                nc.scalar.dma_start(out=uu_sb[k][:, :], in_=uuT[k * 128:(k + 1) * 128, :])
                nc.sync.dma_start(out=xT_sb[k][:, XH:], in_=xT[k * 128:(k + 1) * 128, XH:])
            for h in range(2):
                nc.vector.tensor_copy(comp3(gth[h]), half3(gt_sb[:, :], h))
                nc.vector.tensor_scalar(gtch[h][:, :], gth[h][:, :], -1.0, 1.0,
                                        OP.mult, OP.add)
                nc.vector.tensor_copy(hbfh[h][:, :], hTh[h][:, :])

            ring3 = ring[:, :].rearrange("p (s c) -> p s c", c=SLOT)
            # ub segments of every ring slot, written once: ubH at h*256+128
            for s in range(RING):
                for h in range(2):
                    nc.vector.tensor_copy(
                        ring3[:, s, h * 256 + 128:h * 256 + 256],
                        half3(ub_sb[:, :], h))

            NPRE = 4         # distinct precompute psum buffers

            def precompute_chunk(i, m):
                # xr/xn chunk m for steps i*BLK..(i+1)*BLK into ring block i%2
                slot = (i * MC + m) % NPRE
                ps = ppp.tile([128, BLK * 32], F32, tag=f"pre{slot}",
                              name=f"pre{i}_{m}", padded_shape=[128, 512])
                for k in range(KC):
                    nc.tensor.matmul(
                        ps[:, :],
                        wpre_sb[k][:, m * 128:(m + 1) * 128],
                        xT_sb[k][:, i * BLK * 32:(i + 1) * BLK * 32],
                        start=(k == 0),
                        stop=(k == KC - 1),
                    )
                s0 = (i % NRING) * BLK
                # slot cols for chunk m, half h:
                #   r (m<8):  h*256 + m*16      n (m>=8): 512 + h*128 + (m-8)*16
                src = ps[:, :].rearrange("p (s b) -> p s b", b=32)
                for h in range(2):
                    if m < 8:
                        c0 = h * 256 + m * 16
                    else:
                        c0 = 512 + h * 128 + (m - 8) * 16
                    dst = ring3[:, s0:s0 + BLK, c0:c0 + 16]
                    # out = in + bias (per-partition), f32 psum -> bf16 ring,
                    # split across ACT/DVE so neither queue eats both writes
                    if h == 0:
                        nc.scalar.activation(dst, src[:, :, 0:16],
                                             AF.Identity, bias=bias_sb[:, m:m + 1])
                    else:
                        nc.vector.tensor_scalar(dst, src[:, :, 16:32],
                                                bias_sb[:, m:m + 1], None, OP.add)

            def make_psh(t):
                # PSUM init for step t, emitted one step EARLY so it sits at
                # the head of the ACT queue instead of behind sig/tanh:
                # ps[0:128] = xr_h (+r-biases), ps[128:256] = ub_h.
                s = t % RING
                psh = []
                for h in range(2):
                    ps = psp.tile([128, 256], F32, tag=f"ps{t % 2}{h}",
                                  name=f"ps{t}_{h}", padded_shape=[128, 512])
                    nc.scalar.activation(ps[:, :],
                                         ring3[:, s, h * 256:(h + 1) * 256],
                                         AF.Identity, bias=0.0)
                    psh.append(ps)
                return psh

            def scan_step(t, psh):
                s = t % RING
                # q = gt*h(t-1): only needs the previous h, runs during matmuls
                qh = []
                for h in range(2):
                    q = ew.tile([128, 128], F32, tag=f"q{h}", name=f"q{h}_{t}")
                    nc.vector.tensor_tensor(q[:, :], hTh[h][:, :], gth[h][:, :],
                                            OP.mult)
                    qh.append(q)
                for h in range(2):
                    ps = psh[h]
                    for m in range(MC):
                        col = m * 16 if m < 8 else 128 + (m - 8) * 16
                        for k in range(KC):
                            nc.tensor.matmul(
                                ps[:, col:col + 16],
                                uu_sb[k][:, m * 128:(m + 1) * 128],
                                hbfh[h][:, k * 16:(k + 1) * 16],
                                start=False,
                                stop=(k == KC - 1),
                                skip_group_check=True,
                            )

                # Stage-interleaved emission so the in-order engine queues run
                # chain B one uu-half behind chain A without stalls.
                rt, n2, an, nt = [None, None], [None, None], [None, None], [None, None]
                for h in range(2):
                    rt[h] = ew.tile([128, 128], F32, tag=f"rt{h}", name=f"rt{h}_{t}")
                    nc.scalar.activation(rt[h][:, :], psh[h][:, 0:128], AF.Sigmoid)
                for h in range(2):
                    n2[h] = ew.tile([128, 128], F32, tag=f"n2{h}", name=f"n2{h}_{t}")
                    nc.vector.tensor_tensor(n2[h][:, :], rt[h][:, :],
                                            psh[h][:, 128:256], OP.mult)
                for h in range(2):
                    an[h] = ew.tile([128, 128], F32, tag=f"an{h}", name=f"an{h}_{t}")
                    nc.gpsimd.tensor_tensor(an[h][:, :], n2[h][:, :],
                                            ring3[:, s, 512 + h * 128:512 + (h + 1) * 128],
                                            OP.add)
                for h in range(2):
                    nt[h] = ew.tile([128, 128], F32, tag=f"nt{h}", name=f"nt{h}_{t}")
                    nc.scalar.activation(nt[h][:, :], an[h][:, :], AF.Tanh)
                ph = []
                for h in range(2):
                    p = ew.tile([128, 128], F32, tag=f"p{h}", name=f"p{h}_{t}")
                    nc.vector.tensor_tensor(p[:, :], nt[h][:, :], gtch[h][:, :],
                                            OP.mult)
                    # bf16 h feeds the next step's matmuls
                    nc.vector.tensor_tensor(hbfh[h][:, :], p[:, :], qh[h][:, :],
                                            OP.add)
                    ph.append(p)
                for h in range(2):
                    # f32 h updates in the chain's shadow; kept on DVE so the
                    # next step's q (which reads hTh) serializes naturally
                    # after it instead of stalling mid-queue
                    nc.vector.tensor_tensor(hTh[h][:, :], ph[h][:, :],
                                            qh[h][:, :], OP.add)

            for m in range(MC):
                precompute_chunk(0, m)
            cur_psh = make_psh(0)
            for i in range(NBLK):
                for u in range(BLK):
                    t = i * BLK + u
                    # Next step's PSUM init goes ahead of this step's ACT ops —
                    # except at block boundaries, where it must follow the
                    # ring writes of the block it reads.
                    next_psh = None
                    if u < BLK - 1 and t + 1 < T:
                        next_psh = make_psh(t + 1)
                    scan_step(t, cur_psh)
                    if i + 1 < NBLK:
                        precompute_chunk(i + 1, 2 * u)
                        precompute_chunk(i + 1, 2 * u + 1)
                    if next_psh is None and t + 1 < T:
                        next_psh = make_psh(t + 1)
                    cur_psh = next_psh

            for h in range(2):
                nc.sync.dma_start(out=half3(out[:, :], h), in_=comp3(hTh[h]))

    nc.finalize()
    return nc


def _prep_inputs(x, h0, gt, Wr_w, Wr_b, Ur_w, Ur_b, W_w, W_b, U_w, U_b):
    bf = ml_dtypes.bfloat16
    wpreT = np.ascontiguousarray(
        np.concatenate([Wr_w.T, W_w.T], axis=1)).astype(bf)          # [H, 2048]
    uuT = np.ascontiguousarray(
        np.concatenate([Ur_w.T, U_w.T], axis=1)).astype(bf)          # [H, 2048]
    # biasp[:, m] = per-partition bias for precompute chunk m, added during
    # the psum->ring copy: r-chunks get Wr_b+Ur_b, n-chunks get W_b.
    # ubT = U_b broadcast (copied into every ring slot's ub segments).
    def hmajor_bcast(v):
        return np.ascontiguousarray(
            np.broadcast_to(v.reshape(8, 128).T[:, :, None],
                            (128, 8, 32)).reshape(128, 256))
    bias_cat = np.concatenate([(Wr_b + Ur_b), W_b]).astype(np.float32)
    biasp = np.ascontiguousarray(bias_cat.reshape(MC, 128).T)
    ubT = hmajor_bcast(U_b.astype(np.float32)).astype(bf)

    in_maps = []
    for c in range(NCORES):
        sl = slice(c * BL, (c + 1) * BL)
        x_loc = x[sl]                                  # [32, 128, 1024]
        xT = np.ascontiguousarray(
            x_loc.transpose(2, 1, 0).reshape(H, BT)).astype(bf)
        h0T = np.ascontiguousarray(
            h0[sl].reshape(BL, 8, 128).transpose(2, 1, 0).reshape(128, 256)
        ).astype(np.float32)
        gtT = np.ascontiguousarray(
            np.broadcast_to(gt[sl].reshape(BL)[None, None, :],
                            (128, 8, 32)).reshape(128, 256)).astype(np.float32)
        in_maps.append({
            "xT": xT, "wpreT": wpreT, "uuT": uuT, "biasp": biasp,
            "ubT": ubT, "gtT": gtT, "h0T": h0T,
        })
    return in_maps


def kernel(x, h0, gt, Wr_w, Wr_b, Ur_w, Ur_b, Wz_w, Wz_b, Uz_w, Uz_b,
           W_w, W_b, U_w, U_b, _trace=False, _tmpdir=None):
    x = np.asarray(x, np.float32)
    h0 = np.asarray(h0, np.float32)
    gt = np.asarray(gt, np.float32)
    in_maps = _prep_inputs(x, h0, gt,
                           np.asarray(Wr_w, np.float32), np.asarray(Wr_b, np.float32),
                           np.asarray(Ur_w, np.float32), np.asarray(Ur_b, np.float32),
                           np.asarray(W_w, np.float32), np.asarray(W_b, np.float32),
                           np.asarray(U_w, np.float32), np.asarray(U_b, np.float32))
    if "nc" not in _CACHE:
        _CACHE["nc"] = _build_bass()
    res = run_bass_kernel_spmd(_CACHE["nc"], in_maps, core_ids=list(range(NCORES)),
                               trace=_trace, tmpdir=_tmpdir)
    outs = []
    for c in range(NCORES):
        o = np.asarray(res.results[c]["out"], np.float32)       # [128, 256]
        outs.append(o.reshape(128, 8, BL).transpose(2, 1, 0).reshape(BL, H))
    full = np.concatenate(outs, axis=0)                          # [256, 1024]
    if _trace:
        return full, res
    return full


# revision 29
# speedup vs baseline: 1.0011x; 1.0011x over previous
"""AttnGRU Trainium2 kernel: 8-way data-parallel, H-major dataflow.

v5 over the baseline (same host I/O contract as the baseline):
  - The 32-batch scan is split into two 16-batch half-chains (A/B) with
    fully separate tiles (own PSUM tile, own hbf/h tiles, own ring
    regions), so the tile dep-tracker sees no overlap and the two
    recurrence chains pipeline against each other: while A's
    sigmoid/tanh chain runs, the PE runs B's matmuls, and vice versa.
  - Ring slots are laid out per half: [xrA|ubA | xrB|ubB | xnA|xnB]
    (768 cols), so each half's PSUM init is ONE contiguous identity
    matmul and each chain op reads compact APs.
  - 2-block precompute ring; block i+1 is emitted 2 m-chunks per scan
    step of block i (no PE bursts, WAR pre-satisfied); psum->ring copies
    run on the ACT engine with the bias add fused.
  - Short critical chain per half: q = gt*h runs off-chain during the
    matmuls; chain is sigmoid -> n2 -> an(Pool) -> tanh -> p -> hbf; the
    f32 h update happens in the chain's shadow.

Math per core (B_loc=32, T=128, H=1024):
  xr = x @ Wr_w.T + (Wr_b + Ur_b)      (precomputed, blocked over time)
  xn = x @ W_w.T  + W_b
  per step: rt = sigmoid(xr_t + h @ Ur_w.T)
            nt = tanh(xn_t + rt * (h @ U_w.T + U_b))
            h  = (1-gt)*nt + gt*h

Tiles are H-major: [128 partitions = H-chunk, free = (chunk, batch)].
"""

import numpy as np
import ml_dtypes

import concourse.bass as bass
import concourse.bacc as bacc
import concourse.mybir as mybir
from concourse import tile
from concourse.bass_utils import run_bass_kernel_spmd

B, T, H = 256, 128, 1024
NCORES = 8
BL = B // NCORES          # 32 batch rows per core
BT = BL * T               # 4096 (time-major: col = t*32 + b)
KC = H // 128             # 8 contraction chunks
MC = 2048 // 128          # 16 output chunks ([r | n] concat)
BLK = 8                   # scan steps per precompute block
NBLK = T // BLK           # 16
NRING = 2                 # ring depth in blocks
RING = NRING * BLK        # 16 per-step slots
SLOT = 768                # per-slot cols: [xrA 128|ubA 128|xrB 128|ubB 128|xnA 128|xnB 128]

BF = mybir.dt.bfloat16
F32 = mybir.dt.float32
AF = mybir.ActivationFunctionType
OP = mybir.AluOpType

_CACHE = {}


def _build_bass():
    nc = bacc.Bacc()
    xT = nc.declare_dram_parameter("xT", [H, BT], BF, isOutput=False)
    wpreT = nc.declare_dram_parameter("wpreT", [H, 2048], BF, isOutput=False)
    uuT = nc.declare_dram_parameter("uuT", [H, 2048], BF, isOutput=False)
    biasp = nc.declare_dram_parameter("biasp", [128, MC], F32, isOutput=False)
    ubT = nc.declare_dram_parameter("ubT", [128, 256], BF, isOutput=False)
    gtT = nc.declare_dram_parameter("gtT", [128, 256], F32, isOutput=False)
    h0T = nc.declare_dram_parameter("h0T", [128, 256], F32, isOutput=False)
    out = nc.declare_dram_parameter("out", [128, 256], F32, isOutput=True)

    with tile.TileContext(nc) as tc:
        with (
            tc.tile_pool(name="w", bufs=1) as wp,
            tc.tile_pool(name="ew", bufs=3) as ew,
            tc.tile_pool(name="ps", bufs=1, space="PSUM") as psp,
            tc.tile_pool(name="pp", bufs=1, space="PSUM") as ppp,
        ):
            xT_sb = [wp.tile([128, BT], BF, tag=f"xT{k}", name=f"xT{k}") for k in range(KC)]
            uu_sb = [wp.tile([128, 2048], BF, tag=f"uu{k}", name=f"uu{k}") for k in range(KC)]
            wpre_sb = [wp.tile([128, 2048], BF, tag=f"wp{k}", name=f"wp{k}") for k in range(KC)]
            ring = wp.tile([128, RING * SLOT], BF, tag="ring")
            bias_sb = wp.tile([128, MC], F32, tag="bias")
            ub_sb = wp.tile([128, 256], BF, tag="ub")
            gt_sb = wp.tile([128, 256], F32, tag="gt")
            # per-half (16-batch) compact tiles for the two pipelined chains
            gth = [wp.tile([128, 128], F32, tag=f"gth{h}", name=f"gth{h}") for h in range(2)]
            gtch = [wp.tile([128, 128], F32, tag=f"gtch{h}", name=f"gtch{h}") for h in range(2)]
            hTh = [wp.tile([128, 128], F32, tag=f"hTh{h}", name=f"hTh{h}") for h in range(2)]
            hbfh = [wp.tile([128, 128], BF, tag=f"hbfh{h}", name=f"hbfh{h}") for h in range(2)]

            def half3(ap, h, b=32):
                # [128, (c b)] view, half h -> [128, c, 16] (strided 16-col slices)
                v = ap.rearrange("p (c b) -> p c b", b=b)
                return v[:, :, h * 16:(h + 1) * 16]

            def comp3(tile_):
                # compact [128, 128] tile as [128, c, 16]
                return tile_[:, :].rearrange("p (c b) -> p c b", b=16)

            nc.sync.dma_start(out=gt_sb[:, :], in_=gtT[:, :])
            nc.sync.dma_start(out=bias_sb[:, :], in_=biasp[:, :])
            nc.sync.dma_start(out=ub_sb[:, :], in_=ubT[:, :])
            for h in range(2):
                nc.sync.dma_start(out=comp3(hTh[h]), in_=half3(h0T[:, :], h))
            XH = 2 * BLK * 32        # 512 cols: blocks 0-1
            for k in range(KC):
                nc.scalar.dma_start(out=wpre_sb[k][:, :], in_=wpreT[k * 128:(k + 1) * 128, :])
                nc.sync.dma_start(out=xT_sb[k][:, 0:XH], in_=xT[k * 128:(k + 1) * 128, 0:XH])
            for k in range(KC):
                nc.scalar.dma_start(out=uu_sb[k][:, :], in_=uuT[k * 128:(k + 1) * 128, :])
                nc.sync.dma_start(out=xT_sb[k][:, XH:], in_=xT[k * 128:(k + 1) * 128, XH:])
            for h in range(2):
                nc.vector.tensor_copy(comp3(gth[h]), half3(gt_sb[:, :], h))
                nc.vector.tensor_scalar(gtch[h][:, :], gth[h][:, :], -1.0, 1.0,
                                        OP.mult, OP.add)
                nc.vector.tensor_copy(hbfh[h][:, :], hTh[h][:, :])

            ring3 = ring[:, :].rearrange("p (s c) -> p s c", c=SLOT)
            # ub segments of every ring slot, written once: ubH at h*256+128
            for s in range(RING):
                for h in range(2):
                    nc.vector.tensor_copy(
                        ring3[:, s, h * 256 + 128:h * 256 + 256],
                        half3(ub_sb[:, :], h))

            NPRE = 4         # distinct precompute psum buffers

            def precompute_chunk(i, m):
                # xr/xn chunk m for steps i*BLK..(i+1)*BLK into ring block i%2
                slot = (i * MC + m) % NPRE
                ps = ppp.tile([128, BLK * 32], F32, tag=f"pre{slot}",
                              name=f"pre{i}_{m}", padded_shape=[128, 512])
                for k in range(KC):
                    nc.tensor.matmul(
                        ps[:, :],
                        wpre_sb[k][:, m * 128:(m + 1) * 128],
                        xT_sb[k][:, i * BLK * 32:(i + 1) * BLK * 32],
                        start=(k == 0),
                        stop=(k == KC - 1),
                    )
                s0 = (i % NRING) * BLK
                # slot cols for chunk m, half h:
                #   r (m<8):  h*256 + m*16      n (m>=8): 512 + h*128 + (m-8)*16
                src = ps[:, :].rearrange("p (s b) -> p s b", b=32)
                for h in range(2):
                    if m < 8:
                        c0 = h * 256 + m * 16
                    else:
                        c0 = 512 + h * 128 + (m - 8) * 16
                    dst = ring3[:, s0:s0 + BLK, c0:c0 + 16]
                    # out = in + bias (per-partition), f32 psum -> bf16 ring,
                    # split across ACT/DVE so neither queue eats both writes
                    if h == 0:
                        nc.scalar.activation(dst, src[:, :, 0:16],
                                             AF.Identity, bias=bias_sb[:, m:m + 1])
                    else:
                        nc.vector.tensor_scalar(dst, src[:, :, 16:32],
                                                bias_sb[:, m:m + 1], None, OP.add)

            def make_psh(t):
                # PSUM init for step t, emitted one step EARLY so it sits at
                # the head of the ACT queue instead of behind sig/tanh:
                # ps[0:128] = xr_h (+r-biases), ps[128:256] = ub_h.
                s = t % RING
                psh = []
                for h in range(2):
                    ps = psp.tile([128, 256], F32, tag=f"ps{t % 2}{h}",
                                  name=f"ps{t}_{h}", padded_shape=[128, 512])
                    nc.scalar.activation(ps[:, :],
                                         ring3[:, s, h * 256:(h + 1) * 256],
                                         AF.Identity, bias=0.0)
                    psh.append(ps)
                return psh

            def scan_step(t, psh):
                s = t % RING
                # q = gt*h(t-1): only needs the previous h, runs during matmuls
                qh = []
                for h in range(2):
                    q = ew.tile([128, 128], F32, tag=f"q{h}", name=f"q{h}_{t}")
                    nc.vector.tensor_tensor(q[:, :], hTh[h][:, :], gth[h][:, :],
                                            OP.mult)
                    qh.append(q)
                for h in range(2):
                    ps = psh[h]
                    for m in range(MC):
                        col = m * 16 if m < 8 else 128 + (m - 8) * 16
                        for k in range(KC):
                            nc.tensor.matmul(
                                ps[:, col:col + 16],
                                uu_sb[k][:, m * 128:(m + 1) * 128],
                                hbfh[h][:, k * 16:(k + 1) * 16],
                                start=False,
                                stop=(k == KC - 1),
                                skip_group_check=True,
                            )

                # Stage-interleaved emission so the in-order engine queues run
                # chain B one uu-half behind chain A without stalls.
                rt, n2, an, nt = [None, None], [None, None], [None, None], [None, None]
                for h in range(2):
                    rt[h] = ew.tile([128, 128], F32, tag=f"rt{h}", name=f"rt{h}_{t}")
                    nc.scalar.activation(rt[h][:, :], psh[h][:, 0:128], AF.Sigmoid)
                for h in range(2):
                    n2[h] = ew.tile([128, 128], F32, tag=f"n2{h}", name=f"n2{h}_{t}")
                    nc.vector.tensor_tensor(n2[h][:, :], rt[h][:, :],
                                            psh[h][:, 128:256], OP.mult)
                for h in range(2):
                    an[h] = ew.tile([128, 128], F32, tag=f"an{h}", name=f"an{h}_{t}")
                    nc.vector.tensor_tensor(an[h][:, :], n2[h][:, :],
                                            ring3[:, s, 512 + h * 128:512 + (h + 1) * 128],
                                            OP.add)
                for h in range(2):
                    nt[h] = ew.tile([128, 128], F32, tag=f"nt{h}", name=f"nt{h}_{t}")
                    nc.scalar.activation(nt[h][:, :], an[h][:, :], AF.Tanh)
                ph = []
                for h in range(2):
                    p = ew.tile([128, 128], F32, tag=f"p{h}", name=f"p{h}_{t}")
                    nc.vector.tensor_tensor(p[:, :], nt[h][:, :], gtch[h][:, :],
                                            OP.mult)
                    # bf16 h feeds the next step's matmuls
                    nc.vector.tensor_tensor(hbfh[h][:, :], p[:, :], qh[h][:, :],
                                            OP.add)
                    ph.append(p)
                for h in range(2):
                    # f32 h updates in the chain's shadow; kept on DVE so the
                    # next step's q (which reads hTh) serializes naturally
                    # after it instead of stalling mid-queue
                    nc.vector.tensor_tensor(hTh[h][:, :], ph[h][:, :],
                                            qh[h][:, :], OP.add)

            for m in range(MC):
                precompute_chunk(0, m)
            cur_psh = make_psh(0)
            for i in range(NBLK):
                for u in range(BLK):
                    t = i * BLK + u
                    # Next step's PSUM init goes ahead of this step's ACT ops —
                    # except at block boundaries, where it must follow the
                    # ring writes of the block it reads.
                    next_psh = None
                    if u < BLK - 1 and t + 1 < T:
                        next_psh = make_psh(t + 1)
                    scan_step(t, cur_psh)
                    if i + 1 < NBLK:
                        precompute_chunk(i + 1, 2 * u)
                        precompute_chunk(i + 1, 2 * u + 1)
                    if next_psh is None and t + 1 < T:
                        next_psh = make_psh(t + 1)
                    cur_psh = next_psh

            for h in range(2):
                nc.sync.dma_start(out=half3(out[:, :], h), in_=comp3(hTh[h]))

    nc.finalize()
    return nc


def _prep_inputs(x, h0, gt, Wr_w, Wr_b, Ur_w, Ur_b, W_w, W_b, U_w, U_b):
    bf = ml_dtypes.bfloat16
    wpreT = np.ascontiguousarray(
        np.concatenate([Wr_w.T, W_w.T], axis=1)).astype(bf)          # [H, 2048]
    uuT = np.ascontiguousarray(
        np.concatenate([Ur_w.T, U_w.T], axis=1)).astype(bf)          # [H, 2048]
    # biasp[:, m] = per-partition bias for precompute chunk m, added during
    # the psum->ring copy: r-chunks get Wr_b+Ur_b, n-chunks get W_b.
    # ubT = U_b broadcast (copied into every ring slot's ub segments).
    def hmajor_bcast(v):
        return np.ascontiguousarray(
            np.broadcast_to(v.reshape(8, 128).T[:, :, None],
                            (128, 8, 32)).reshape(128, 256))
    bias_cat = np.concatenate([(Wr_b + Ur_b), W_b]).astype(np.float32)
    biasp = np.ascontiguousarray(bias_cat.reshape(MC, 128).T)
    ubT = hmajor_bcast(U_b.astype(np.float32)).astype(bf)

    in_maps = []
    for c in range(NCORES):
        sl = slice(c * BL, (c + 1) * BL)
        x_loc = x[sl]                                  # [32, 128, 1024]
        xT = np.ascontiguousarray(
            x_loc.transpose(2, 1, 0).reshape(H, BT)).astype(bf)
        h0T = np.ascontiguousarray(
            h0[sl].reshape(BL, 8, 128).transpose(2, 1, 0).reshape(128, 256)
        ).astype(np.float32)
        gtT = np.ascontiguousarray(
            np.broadcast_to(gt[sl].reshape(BL)[None, None, :],
                            (128, 8, 32)).reshape(128, 256)).astype(np.float32)
        in_maps.append({
            "xT": xT, "wpreT": wpreT, "uuT": uuT, "biasp": biasp,
            "ubT": ubT, "gtT": gtT, "h0T": h0T,
        })
    return in_maps


def kernel(x, h0, gt, Wr_w, Wr_b, Ur_w, Ur_b, Wz_w, Wz_b, Uz_w, Uz_b,
           W_w, W_b, U_w, U_b, _trace=False, _tmpdir=None):
    x = np.asarray(x, np.float32)
    h0 = np.asarray(h0, np.float32)
    gt = np.asarray(gt, np.float32)
    in_maps = _prep_inputs(x, h0, gt,
                           np.asarray(Wr_w, np.float32), np.asarray(Wr_b, np.float32),
                           np.asarray(Ur_w, np.float32), np.asarray(Ur_b, np.float32),
                           np.asarray(W_w, np.float32), np.asarray(W_b, np.float32),
                           np.asarray(U_w, np.float32), np.asarray(U_b, np.float32))
    if "nc" not in _CACHE:
        _CACHE["nc"] = _build_bass()
    res = run_bass_kernel_spmd(_CACHE["nc"], in_maps, core_ids=list(range(NCORES)),
                               trace=_trace, tmpdir=_tmpdir)
    outs = []
    for c in range(NCORES):
        o = np.asarray(res.results[c]["out"], np.float32)       # [128, 256]
        outs.append(o.reshape(128, 8, BL).transpose(2, 1, 0).reshape(BL, H))
    full = np.concatenate(outs, axis=0)                          # [256, 1024]
    if _trace:
        return full, res
    return full
